# revision 1
# baseline (speedup 1.0000x reference)
"""Multi-head attention (B=1, S=4096, D=1024, H=16) on 8 TRN2 NeuronCores.

Strategy (head-sharded attention + AllToAll context exchange):
  - Host: compact K/V to the unmasked key positions (mask==0 keys contribute
    exactly 0 to softmax numerator and denominator, since the reference's
    -1e9 masking underflows exp to 0.0), transpose activations/weights to
    feature-major, cast matmul operands to bf16.
  - Phase A: core m computes K^T and V projections for its 2 heads over all
    compacted positions; results stay in SBUF (no gather needed).
  - Phase B: Q projection for the same 2 heads over ALL 4096 queries.
  - Phase C: attention for the 2 heads x 4096 queries: scores^T =
    K^T-chunk.T @ Q^T in PSUM ([k,q] layout, per-head via matmul
    tile_position row groups), exp on ScalarE straight out of PSUM (padding
    bias folded into the per-partition activation bias), P@V with a
    ones-augmented V (row 64 = softmax denominators), reciprocal + K=1
    broadcast matmul + multiply to normalize. Per-head context goes to DRAM
    sliced by query block.
  - AllToAll (one per head, 2 MiB, the first overlaps the second head's
    compute) converts head-sharding to query-sharding: afterwards core m
    holds all 16 heads' context for its own 512 queries.
  - Phase D: output projection of the core's 512 rows. The host just
    concatenates the 8 row-slices.
"""

import numpy as np
import ml_dtypes

import concourse.bacc as bacc
import concourse.mybir as mybir
import concourse.tile as tile
from concourse.bass_utils import run_bass_kernel_spmd

HEADS = 16
D = 1024
DH = 64
S = 4096
N_CORES = 8
SQ = S // N_CORES          # query rows owned per core (output sharding)
HPC = HEADS // N_CORES     # heads per core
BF16 = mybir.dt.bfloat16
F32 = mybir.dt.float32

NEG_BIG = -3840.0          # exp(-3840) == 0.0 exactly in fp32
EXP_GROUP = 3              # k-chunks (PSUM banks) per exp activation op


def _bf16(x):
    return np.ascontiguousarray(x.astype(ml_dtypes.bfloat16))


def build_program(n_pad, kc_real, kc_mixed, repeat=1, ablate=()):
    """Build the 8-core SPMD program.

    n_pad: padded compacted key count (multiple of 512).
    kc_real: number of leading k-chunks (of 128) with no padding.
    kc_mixed: 1 if a chunk straddles n (it gets a per-partition bias column
    on its exp); chunks past kc_real + kc_mixed are fully padded and get a
    constant NEG_BIG bias.
    """
    KC = n_pad // 128
    NG = n_pad // 512
    QC = S // 512            # query column groups (whole sequence)
    nc = bacc.Bacc("TRN2", target_bir_lowering=False, debug=False,
                   num_devices=N_CORES)

    # ---- I/O ----  (all bf16 unless noted; feature-major activations)
    qT = nc.dram_tensor("qT", [D, S], BF16, kind="ExternalInput")
    kcT = nc.dram_tensor("kcT", [D, n_pad], BF16, kind="ExternalInput")
    vcT = nc.dram_tensor("vcT", [D, n_pad], BF16, kind="ExternalInput")
    wqT = nc.dram_tensor("wqT", [D, HPC * DH], BF16, kind="ExternalInput")
    wkT = nc.dram_tensor("wkT", [D, HPC * DH], BF16, kind="ExternalInput")
    wvT = nc.dram_tensor("wvT", [D, HPC * DH], BF16, kind="ExternalInput")
    woT = nc.dram_tensor("woT", [D, D], BF16, kind="ExternalInput")
    bq_m = nc.dram_tensor("bq_m", [128, 1], F32, kind="ExternalInput")
    bk_m = nc.dram_tensor("bk_m", [128, 1], F32, kind="ExternalInput")
    bv_m = nc.dram_tensor("bv_m", [1, HPC * DH], BF16, kind="ExternalInput")
    bo_r = nc.dram_tensor("bo_r", [1, D], BF16, kind="ExternalInput")
    # per k-chunk exp bias column (0 for real keys, NEG_BIG for padding)
    pbias = nc.dram_tensor("pbias", [128, KC], F32, kind="ExternalInput")
    out = nc.dram_tensor("out", [SQ, D], F32, kind="ExternalOutput")

    with tile.TileContext(nc) as tc:
        for _rep in range(repeat):
            with (
                tc.tile_pool(name="dram", bufs=1, space="DRAM") as dram,
                tc.tile_pool(name="consts", bufs=1) as consts,
                tc.tile_pool(name="persist", bufs=1) as persist,
            ):
                # per-head A2A buffers: shard q-block -> [64 feats, 512 q]
                a2a_in = [dram.tile([N_CORES, 64, 512], BF16, name=f"a2i{j}")
                          for j in range(HPC)]
                a2a_out = [dram.tile([N_CORES, 64, 512], BF16, name=f"a2o{j}")
                           for j in range(HPC)]

                ones_bf = consts.tile([1, 128], BF16)
                nc.vector.memset(ones_bf[:], 1.0)
                ones_f = consts.tile([1, 64], F32)
                nc.vector.memset(ones_f[:], 1.0)
                bq_sb = consts.tile([128, 1], F32)
                nc.sync.dma_start(bq_sb[:], bq_m[:])
                bk_sb = consts.tile([128, 1], F32)
                nc.sync.dma_start(bk_sb[:], bk_m[:])
                bv_sb = consts.tile([1, HPC * DH], BF16)
                nc.sync.dma_start(bv_sb[:], bv_m[:])
                bo_sb = consts.tile([1, D], BF16)
                nc.sync.dma_start(bo_sb[:], bo_r[:])
                pb_sb = consts.tile([128, KC], F32)
                nc.sync.dma_start(pb_sb[:], pbias[:])

                kT_all = persist.tile([128, n_pad], BF16)
                wq_sb = persist.tile([128, 8, HPC * DH], BF16)
                q0_in = persist.tile([128, 8, 512], BF16)
                # v layout: [n-part, k-chunk, head, DH+1]; col DH == ones
                v_all = persist.tile([128, KC, HPC, DH + 1], BF16)
                q_pair = persist.tile([128, QC, 512], BF16)

                # ---------- Phase A: K/V projection (own 2 heads) ----------
                with (
                    tc.tile_pool(name="a_w", bufs=1) as a_w,
                    tc.tile_pool(name="a_in", bufs=1) as a_in,
                    tc.tile_pool(name="a_ps", bufs=2, space="PSUM") as a_ps,
                ):
                    wk_sb = a_w.tile([128, 8, HPC * DH], BF16)
                    wv_sb = a_w.tile([128, 8, HPC * DH], BF16)
                    for c in range(8):
                        nc.sync.dma_start(wk_sb[:, c, :],
                                          wkT[c * 128:(c + 1) * 128, :])
                        nc.sync.dma_start(wv_sb[:, c, :],
                                          wvT[c * 128:(c + 1) * 128, :])
                        nc.sync.dma_start(wq_sb[:, c, :],
                                          wqT[c * 128:(c + 1) * 128, :])
                        nc.sync.dma_start(q0_in[:, c, :],
                                          qT[c * 128:(c + 1) * 128, 0:512])
                    nc.vector.memset(v_all[:, :, :, DH:DH + 1], 1.0)

                    # one fat contiguous DMA per 128-row chunk
                    kin = a_in.tile([128, 8, n_pad], BF16)
                    vin = a_in.tile([128, 8, n_pad], BF16)
                    for c in range(8):
                        nc.sync.dma_start(kin[:, c, :],
                                          kcT[c * 128:(c + 1) * 128, :])
                    for c in range(8):
                        nc.scalar.dma_start(vin[:, c, :],
                                            vcT[c * 128:(c + 1) * 128, :])

                    ps_q0 = a_ps.tile([128, 512], F32, tag="psk")
                    for c in range(8):
                        nc.tensor.matmul(ps_q0[:], wq_sb[:, c, :],
                                         q0_in[:, c, :],
                                         start=(c == 0), stop=(c == 7))
                    nc.vector.tensor_scalar_add(q_pair[:, 0, :], ps_q0[:],
                                                bq_sb[:])

                    for g in range(NG):
                        ns = slice(g * 512, (g + 1) * 512)
                        ps_k = a_ps.tile([128, 512], F32, tag="psk")
                        for c in range(8):
                            nc.tensor.matmul(ps_k[:], wk_sb[:, c, :],
                                             kin[:, c, ns],
                                             start=(c == 0), stop=(c == 7))
                        nc.vector.tensor_scalar_add(kT_all[:, ns], ps_k[:],
                                                    bk_sb[:])
                    for g in range(NG):
                        for jj in range(4):
                            kc = g * 4 + jj
                            ks = slice(kc * 128, (kc + 1) * 128)
                            ps_v = a_ps.tile([128, HPC * DH], F32, tag="psv")
                            for c in range(8):
                                nc.tensor.matmul(
                                    ps_v[:], vin[:, c, ks],
                                    wv_sb[:, c, :], start=(c == 0), stop=False)
                            nc.tensor.matmul(ps_v[:], ones_bf[:, :128],
                                             bv_sb[:], start=False, stop=True)
                            for j in range(HPC):
                                nc.vector.tensor_copy(
                                    v_all[:, kc, j, 0:DH],
                                    ps_v[:, j * DH:(j + 1) * DH])

                # ---------- Phase B folded into C: q blocks on demand ----
                qin = persist.tile([128, 8, S - 512], BF16)
                for c in range(8):
                    nc.gpsimd.dma_start(qin[:, c, :],
                                        qT[c * 128:(c + 1) * 128, 512:])

                # ---------- Phase C: attention for own 2 heads ----------
                # wo is loaded early so phase D's weights are resident
                wo_sb2 = persist.tile([128, N_CORES, D], BF16)
                for b in range(N_CORES):
                    nc.gpsimd.dma_start(wo_sb2[:, b, :],
                                        woT[b * 128:(b + 1) * 128, :])
                with (
                    tc.tile_pool(name="c_exp", bufs=3) as c_exp,
                    tc.tile_pool(name="c_misc", bufs=3) as c_misc,
                    tc.tile_pool(name="c_ps_s", bufs=2, space="PSUM") as c_ps_s,
                    tc.tile_pool(name="c_ps_c", bufs=2, space="PSUM") as c_ps_c,
                ):
                    for j in range(HPC):
                        pj = slice(64 * j, 64 * (j + 1))
                        for qc in range(QC):
                            if j == 0 and qc + 1 < QC:
                                qs = slice(qc * 512, (qc + 1) * 512)
                                ps_q = c_ps_s.tile([128, EXP_GROUP, 512], F32,
                                                   tag="s", name=f"psq{qc}")
                                for c in range(8):
                                    nc.tensor.matmul(
                                        ps_q[:, 0, :], wq_sb[:, c, :],
                                        qin[:, c, qs],
                                        start=(c == 0), stop=(c == 7))
                                nc.vector.tensor_scalar_add(
                                    q_pair[:, qc + 1, :], ps_q[:, 0, :],
                                    bq_sb[:])
                            ps_ctx = c_ps_c.tile([DH + 1, 512], F32, tag="ctx")
                            rhs_q = q_pair[pj, qc, :]
                            c0 = 0
                            while c0 < KC:
                                gn = min(EXP_GROUP, KC - c0)
                                ps_s = c_ps_s.tile([128, EXP_GROUP, 512], F32,
                                                   tag="s")
                                if "noscore" not in ablate:
                                    for cc in range(gn):
                                        lc = c0 + cc
                                        nc.tensor.matmul(
                                            ps_s[:, cc, :],
                                            kT_all[pj,
                                                   lc * 128:(lc + 1) * 128],
                                            rhs_q, start=True, stop=True,
                                            tile_position=(64 * j, 0))
                                exp_sb = c_exp.tile([128, EXP_GROUP, 512],
                                                    BF16, tag="e")
                                if "noexp" in ablate:
                                    nc.vector.tensor_copy(exp_sb[:, 0:1, 0:1],
                                                          ps_s[:, 0:1, 0:1])
                                elif c0 + gn <= kc_real:
                                    nc.scalar.activation(
                                        exp_sb[:, 0:gn, :], ps_s[:, 0:gn, :],
                                        mybir.ActivationFunctionType.Exp,
                                        bias=0.0, scale=0.125)
                                else:
                                    for cc in range(gn):
                                        nc.scalar.activation(
                                            exp_sb[:, cc, :], ps_s[:, cc, :],
                                            mybir.ActivationFunctionType.Exp,
                                            bias=pb_sb[:, c0 + cc:c0 + cc + 1],
                                            scale=0.125)
                                if "nopv" not in ablate:
                                    for cc in range(gn):
                                        lc = c0 + cc
                                        nc.tensor.matmul(
                                            ps_ctx[:], v_all[:, lc, j, :],
                                            exp_sb[:, cc, :],
                                            start=(lc == 0),
                                            stop=(lc == KC - 1))
                                c0 += gn

                            if "nonorm" in ablate:
                                continue
                            recip = c_misc.tile([1, 512], F32, tag="recip")
                            nc.vector.reciprocal(recip[:],
                                                 ps_ctx[DH:DH + 1, :])
                            ps_bc = c_ps_c.tile([64, 512], F32, tag="ctx")
                            nc.tensor.matmul(ps_bc[:], ones_f[:], recip[:],
                                             start=True, stop=True)
                            rec_bc = c_misc.tile([64, 512], F32, tag="rbc")
                            nc.vector.tensor_copy(rec_bc[:], ps_bc[:])
                            ctx_sb = c_misc.tile([64, 512], BF16, tag="ctxs")
                            nc.vector.tensor_mul(ctx_sb[:], ps_ctx[0:64, :],
                                                 rec_bc[:])
                            nc.sync.dma_start(a2a_in[j][qc], ctx_sb[:])

                        if "cclocal" in ablate:
                            nc.sync.dma_start(a2a_out[j][:], a2a_in[j][:])
                        else:
                            nc.gpsimd.collective_compute(
                                "AllToAll", mybir.AluOpType.bypass,
                                replica_groups=[list(range(N_CORES))],
                                ins=[a2a_in[j].opt()],
                                outs=[a2a_out[j].opt()])

                # ---------- Phase D: output projection (own 512 rows) ----------
                if "noD" in ablate:
                    continue
                with (
                    tc.tile_pool(name="d_w", bufs=1) as d_w,
                    tc.tile_pool(name="d_out", bufs=2) as d_out,
                    tc.tile_pool(name="d_ps", bufs=4, space="PSUM") as d_ps,
                ):
                    # heads of equal j stacked in pairs on partitions (K=128);
                    # D0 (j=0 pairs) depends only on A2A_0 and overlaps A2A_1.
                    ctx_p = [d_w.tile([128, 4, 512], BF16, name=f"cxp{j}")
                             for j in range(HPC)]
                    o_acc = d_w.tile([128, SQ // 128, D], F32)
                    for j in range(HPC):
                        ev = a2a_out[j].rearrange("(a two) p q -> a two p q",
                                                  two=2)
                        nc.sync.dma_start(
                            ctx_p[j][0:64, :, :],
                            ev[:, 0].rearrange("a p q -> p a q"))
                        nc.sync.dma_start(
                            ctx_p[j][64:128, :, :],
                            ev[:, 1].rearrange("a p q -> p a q"))
                        for qc in range(SQ // 128):
                            for eh in range(2):
                                es = slice(eh * 512, (eh + 1) * 512)
                                ps_o = d_ps.tile([128, 512], F32, tag="pso")
                                for a in range(4):
                                    nc.tensor.matmul(
                                        ps_o[:],
                                        ctx_p[j][:, a,
                                                 qc * 128:(qc + 1) * 128],
                                        wo_sb2[:, 4 * j + a, es],
                                        start=(a == 0),
                                        stop=(j == 1 and a == 3))
                                if j == 0:
                                    nc.tensor.matmul(
                                        ps_o[:], ones_bf[:, 0:128],
                                        bo_sb[:, es], start=False, stop=True)
                                    nc.vector.tensor_copy(
                                        o_acc[:, qc, es], ps_o[:])
                                else:
                                    o_sb = d_out.tile([128, 512], F32,
                                                      tag="osb")
                                    nc.vector.tensor_add(
                                        o_sb[:], o_acc[:, qc, es], ps_o[:])
                                    nc.sync.dma_start(
                                        out[qc * 128:(qc + 1) * 128, es],
                                        o_sb[:])

    nc.compile()
    return nc


def prepare(query, key, value, mask, Wq, bq, Wk, bk, Wv, bv, Wo, bo):
    """Host-side sharding/preprocessing + program build. Returns the compiled
    Bass program and the per-core input maps."""
    query = np.asarray(query)
    key = np.asarray(key)
    value = np.asarray(value)
    mask = np.asarray(mask)
    Wq, bq = np.asarray(Wq), np.asarray(bq)
    Wk, bk = np.asarray(Wk), np.asarray(bk)
    Wv, bv = np.asarray(Wv), np.asarray(bv)
    Wo, bo = np.asarray(Wo), np.asarray(bo)

    idx = np.nonzero(mask.reshape(-1) != 0)[0]
    n = int(idx.size)
    n_pad = max(512, ((n + 511) // 512) * 512)
    KC = n_pad // 128
    kc_real = n // 128
    kc_mixed = 1 if n % 128 else 0

    key_c = np.zeros((n_pad, D), np.float32)
    val_c = np.zeros((n_pad, D), np.float32)
    key_c[:n] = key[0, idx, :]
    val_c[:n] = value[0, idx, :]

    kcT_np = _bf16(key_c.T)
    vcT_np = _bf16(val_c.T)
    qT_np = _bf16(query[0].T)
    wqT_np = _bf16(Wq.T)
    wkT_np = _bf16(Wk.T)
    wvT_np = _bf16(Wv.T)
    woT_r = Wo.T
    slots = []
    for j in range(HPC):
        for a in range(4):
            hA, hB = 4 * a + j, 4 * a + 2 + j
            slots.append(woT_r[64 * hA:64 * hA + 64, :])
            slots.append(woT_r[64 * hB:64 * hB + 64, :])
    woT_np = _bf16(np.concatenate(slots, axis=0))
    bo_r_np = _bf16(bo.reshape(1, D))

    pb_np = np.zeros((128, KC), np.float32)
    flat = np.full(n_pad, NEG_BIG, np.float32)
    flat[:n] = 0.0
    pb_np[:] = flat.reshape(KC, 128).T

    nc = build_program(n_pad, kc_real, kc_mixed)

    in_maps = []
    for m in range(N_CORES):
        sl = slice(m * 128, (m + 1) * 128)
        in_maps.append({
            "qT": qT_np,
            "kcT": kcT_np,
            "vcT": vcT_np,
            "wqT": np.ascontiguousarray(wqT_np[:, sl]),
            "wkT": np.ascontiguousarray(wkT_np[:, sl]),
            "wvT": np.ascontiguousarray(wvT_np[:, sl]),
            "woT": woT_np,
            "bq_m": np.ascontiguousarray(
                bq[sl].reshape(128, 1).astype(np.float32)),
            "bk_m": np.ascontiguousarray(
                bk[sl].reshape(128, 1).astype(np.float32)),
            "bv_m": _bf16(bv[sl].reshape(1, 128)),
            "bo_r": bo_r_np,
            "pbias": pb_np,
        })

    return {"nc": nc, "in_maps": in_maps, "n": n, "n_pad": n_pad}


def kernel(query, key, value, mask, Wq, bq, Wk, bk, Wv, bv, Wo, bo,
           _trace=False, _result_box=None):
    prep = prepare(query, key, value, mask, Wq, bq, Wk, bk, Wv, bv, Wo, bo)
    res = run_bass_kernel_spmd(prep["nc"], prep["in_maps"],
                               list(range(N_CORES)), trace=_trace)
    if _result_box is not None:
        _result_box.append(res)

    out = np.concatenate([res.results[m]["out"] for m in range(N_CORES)],
                         axis=0)
    return out.reshape(1, S, D).astype(np.float32)



# revision 11
# speedup vs baseline: 1.2823x; 1.2823x over previous
"""Multi-head attention (B=1, S=4096, D=1024, H=16) on 8 TRN2 NeuronCores.

Strategy (head-sharded attention + AllToAll context exchange), v2:
  - Host: compact K/V to the unmasked key positions (mask==0 keys contribute
    exactly 0 to softmax numerator and denominator since exp underflows),
    re-layout activations/weights partition-major, cast matmul operands bf16.
    No padding of the key count: the last 128-chunk is partial (M=44).
  - Core m owns heads 2m, 2m+1.  K projection -> kT_all [128(2h x 64dh), n];
    V projection -> v_all [128 keys, chunk, head, 65] (col 64 = ones, so the
    PV matmul also produces softmax denominators).
  - Attention per (head, qblock of 512 queries): scores^T chunks [128k, 512q]
    in PSUM groups of 3 banks, exp on ScalarE (scale=1/8), then PV with the
    exp tile as the *stationary* operand: out ctx [128 q, 65] per 128-query
    sub-block -- the moving dim is only 65 wide, which the PE cost model
    (cycles ~ moving size) makes ~2x cheaper than the [65, 512] orientation.
  - Normalize with per-partition reciprocal (DVE tensor_scalar), transpose
    ctx back to [64f, 512q] on the PE (identity matmul), stage and DMA to the
    per-head AllToAll buffer.  One AllToAll per head; the first overlaps the
    second head's compute.
  - Phase D: output projection of the core's own 512 query rows (query-
    sharded after the AllToAll), bias via ones-row matmul.
"""

import numpy as np
import ml_dtypes

import concourse.bacc as bacc
import concourse.mybir as mybir
import concourse.tile as tile
from concourse.bass_utils import run_bass_kernel_spmd

HEADS = 16
D = 1024
DH = 64
S = 4096
N_CORES = 8
SQ = S // N_CORES          # query rows owned per core (output sharding)
HPC = HEADS // N_CORES     # heads per core
QC = S // 512              # 512-query blocks over the whole sequence
BF16 = mybir.dt.bfloat16
F32 = mybir.dt.float32
EXP_GROUP = 3              # k-chunks (PSUM banks) per exp activation op


def _bf16(x):
    return np.ascontiguousarray(np.asarray(x).astype(ml_dtypes.bfloat16))


def build_program(n, debug=False):
    """Build the 8-core SPMD program for n (unpadded) compacted keys."""
    KC = (n + 127) // 128                    # 128-key chunks, last partial
    groups = [(c0, min(c0 + EXP_GROUP, KC)) for c0 in range(0, KC, EXP_GROUP)]
    mc = [min(128, n - 128 * c) for c in range(KC)]   # keys in chunk c
    # key-load column groups of <=512 for the projection pipeline
    kgs = [(g0, min(g0 + 512, n)) for g0 in range(0, n, 512)]

    nc = bacc.Bacc("TRN2", target_bir_lowering=False, debug=False,
                   num_devices=N_CORES)

    # ---- I/O ----  (partition-major [128, 8, cols] layouts, bf16)
    q_p = nc.dram_tensor("q_p", [128, 8, S], BF16, kind="ExternalInput")
    kc_p = nc.dram_tensor("kc_p", [128, 8, n], BF16, kind="ExternalInput")
    vc_p = nc.dram_tensor("vc_p", [128, 8, n], BF16, kind="ExternalInput")
    wq_p = nc.dram_tensor("wq_p", [128, 8, 128], BF16, kind="ExternalInput")
    wk_p = nc.dram_tensor("wk_p", [128, 8, 128], BF16, kind="ExternalInput")
    wv_p = nc.dram_tensor("wv_p", [128, 8, 128], BF16, kind="ExternalInput")
    wo_p = nc.dram_tensor("wo_p", [128, 8, D], BF16, kind="ExternalInput")
    bq_m = nc.dram_tensor("bq_m", [128, 1], F32, kind="ExternalInput")
    bk_m = nc.dram_tensor("bk_m", [128, 1], F32, kind="ExternalInput")
    bv_r = nc.dram_tensor("bv_r", [1, 128], BF16, kind="ExternalInput")
    bo_r = nc.dram_tensor("bo_r", [1, D], BF16, kind="ExternalInput")
    ident = nc.dram_tensor("ident", [128, 128], BF16, kind="ExternalInput")
    out = nc.dram_tensor("out", [SQ, D], F32, kind="ExternalOutput")
    if debug:
        dbg_kT = nc.dram_tensor("dbg_kT", [128, n], BF16,
                                kind="ExternalOutput")
        dbg_v = nc.dram_tensor("dbg_v", [128, KC, HPC, DH + 1], BF16,
                               kind="ExternalOutput")
        dbg_q = nc.dram_tensor("dbg_q", [128, QC, 512], BF16,
                               kind="ExternalOutput")
        dbg_ctx = nc.dram_tensor("dbg_ctx", [128, 4, DH], BF16,
                                 kind="ExternalOutput")
        dbg_st = nc.dram_tensor("dbg_st", [64, 512], BF16,
                                kind="ExternalOutput")
        dbg_ex = nc.dram_tensor("dbg_ex", [128, EXP_GROUP, 512], BF16,
                                kind="ExternalOutput")
        ngroups = (KC + EXP_GROUP - 1) // EXP_GROUP
        dbg_ex6 = nc.dram_tensor("dbg_ex6", [128, ngroups, EXP_GROUP, 512],
                                 BF16, kind="ExternalOutput")
        dbg_num = nc.dram_tensor("dbg_num", [128, 4, DH + 1], F32,
                                 kind="ExternalOutput")

    with tile.TileContext(nc) as tc:
        with (
            tc.tile_pool(name="dram", bufs=1, space="DRAM") as dram,
            tc.tile_pool(name="consts", bufs=1) as consts,
            tc.tile_pool(name="persist", bufs=1) as persist,
            tc.tile_pool(name="qld", bufs=2) as qld,
            tc.tile_pool(name="c_exp", bufs=3) as c_exp,
            tc.tile_pool(name="c_misc", bufs=2) as c_misc,
            tc.tile_pool(name="ps_s", bufs=2, space="PSUM") as ps_s,
            tc.tile_pool(name="ps_aux", bufs=1, space="PSUM") as ps_aux,
        ):
            # per-head A2A buffers: dest qblock -> [64 feats, 512 q]
            a2a_in = [dram.tile([N_CORES, 64, 512], BF16, name=f"a2i{j}")
                      for j in range(HPC)]
            a2a_out = [dram.tile([N_CORES, 64, 512], BF16, name=f"a2o{j}")
                       for j in range(HPC)]

            # ---- consts / weights ----
            wq_sb = consts.tile([128, 8, 128], BF16)
            nc.sync.dma_start(wq_sb[:], wq_p[:])
            wk_sb = consts.tile([128, 8, 128], BF16)
            nc.sync.dma_start(wk_sb[:], wk_p[:])
            wv_sb = consts.tile([128, 8, 128], BF16)
            nc.sync.dma_start(wv_sb[:], wv_p[:])
            bq_sb = consts.tile([128, 1], F32)
            nc.sync.dma_start(bq_sb[:], bq_m[:])
            bk_sb = consts.tile([128, 1], F32)
            nc.sync.dma_start(bk_sb[:], bk_m[:])
            bv_sb = consts.tile([1, 128], BF16)
            nc.sync.dma_start(bv_sb[:], bv_r[:])
            bo_sb = consts.tile([1, D], BF16)
            nc.sync.dma_start(bo_sb[:], bo_r[:])
            id_sb = consts.tile([128, 128], BF16)
            nc.sync.dma_start(id_sb[:], ident[:])
            ones_bf = consts.tile([1, 128], BF16)
            nc.vector.memset(ones_bf[:], 1.0)

            # ---- persistent state ----
            kT_all = persist.tile([128, n], BF16)
            v_all = persist.tile([128, KC, HPC, DH + 1], BF16)
            q_pair = persist.tile([128, QC, 512], BF16)
            kin = persist.tile([128, 8, n], BF16)
            vin = persist.tile([128, 8, n], BF16)
            wo_sb = persist.tile([128, 8, D], BF16)
            o_acc = persist.tile([128, SQ // 128, D], F32)
            if mc[-1] < 128:
                # partial last chunk: the PE contraction tile rounds up past
                # the real key count, so the tail rows must multiply to zero
                nc.vector.memset(v_all[:, KC - 1, :, :], 0.0)
                nc.vector.memset(v_all[:, 0:KC - 1, :, DH:DH + 1], 1.0)
                nc.vector.memset(v_all[0:mc[-1], KC - 1, :, DH:DH + 1], 1.0)
            else:
                nc.vector.memset(v_all[:, :, :, DH:DH + 1], 1.0)

            # ---- input loads (SP queue; staggered for early start) ----
            g0, g1 = kgs[0]
            nc.sync.dma_start(kin[:, :, g0:g1], kc_p[:, :, g0:g1])
            q0 = qld.tile([128, 8, 512], BF16, name="qt0", tag="q")
            nc.sync.dma_start(q0[:], q_p[:, :, 0:512])
            nc.sync.dma_start(vin[:, :, g0:g1], vc_p[:, :, g0:g1])
            if len(kgs) > 1:
                r0 = kgs[1][0]
                nc.sync.dma_start(kin[:, :, r0:n], kc_p[:, :, r0:n])
                nc.sync.dma_start(vin[:, :, r0:n], vc_p[:, :, r0:n])
            nc.scalar.dma_start(wo_sb[:], wo_p[:])

            # ---- phase A: K/V projections for the core's 2 heads ----
            # K: kT_all[128 feats, keys], per 512-key group
            for (g0, g1) in kgs:
                kn = g1 - g0
                ps_k = ps_s.tile([128, EXP_GROUP, 512], F32, tag="s",
                                 name=f"psk{g0}")
                for c in range(8):
                    nc.tensor.matmul(ps_k[:, 0, 0:kn], wk_sb[:, c, :],
                                     kin[:, c, g0:g1],
                                     start=(c == 0), stop=(c == 7))
                nc.vector.tensor_scalar_add(kT_all[:, g0:g1],
                                            ps_k[:, 0, 0:kn], bk_sb[:])
            # V: v_all[keys, chunk, head, 0:64], per 128-key chunk
            for c in range(KC):
                m = mc[c]
                ks = slice(128 * c, 128 * c + m)
                ps_v = ps_s.tile([128, EXP_GROUP, 512], F32, tag="s",
                                 name=f"psv{c}")
                pv = ps_v[0:m, 0, 0:128]
                for cc in range(8):
                    nc.tensor.matmul(pv, vin[:, cc, ks], wv_sb[:, cc, :],
                                     start=(cc == 0), stop=False)
                nc.tensor.matmul(pv, ones_bf[:, 0:m], bv_sb[:],
                                 start=False, stop=True)
                nc.vector.tensor_copy(
                    v_all[0:m, c, :, 0:DH],
                    pv.rearrange("p (j f) -> p j f", j=HPC))

            # Q projection for qblock 0
            ps_q = ps_s.tile([128, EXP_GROUP, 512], F32, tag="s", name="psq0")
            for c in range(8):
                nc.tensor.matmul(ps_q[:, 0, :], wq_sb[:, c, :], q0[:, c, :],
                                 start=(c == 0), stop=(c == 7))
            nc.vector.tensor_scalar_add(q_pair[:, 0, :], ps_q[:, 0, :],
                                        bq_sb[:])

            # ---- phase C: attention (head-major; qblock inner) ----
            for j in range(HPC):
                pj = slice(64 * j, 64 * (j + 1))
                for qb in range(QC):
                    if j == 0 and qb + 1 < QC:
                        qn = qld.tile([128, 8, 512], BF16, name=f"qt{qb + 1}",
                                      tag="q")
                        nc.sync.dma_start(
                            qn[:], q_p[:, :, 512 * (qb + 1):512 * (qb + 2)])
                    ps_ctx = ps_aux.tile([128, 4, DH + 1], F32, tag="ctx")
                    for gi, (c0, c1) in enumerate(groups):
                        gn = c1 - c0
                        ps = ps_s.tile([128, EXP_GROUP, 512], F32, tag="s")
                        for c in range(c0, c1):
                            m = mc[c]
                            nc.tensor.matmul(
                                ps[0:m, c - c0, :],
                                kT_all[pj, 128 * c:128 * c + m],
                                q_pair[pj, qb, :], start=True, stop=True,
                                tile_position=(64 * j, 0))
                        ex = c_exp.tile([128, EXP_GROUP, 512], BF16, tag="e")
                        nc.scalar.activation(
                            ex[:, 0:gn, :], ps[:, 0:gn, :],
                            mybir.ActivationFunctionType.Exp,
                            bias=0.0, scale=0.125)
                        if debug and j == 0 and qb == 0 and gi == 0:
                            nc.sync.dma_start(dbg_ex[:], ex[:])
                        if debug and j == 0 and qb == 0:
                            nc.sync.dma_start(dbg_ex6[:, gi, 0:gn, :],
                                              ex[:, 0:gn, :])
                        for c in range(c0, c1):
                            m = mc[c]
                            for s4 in range(4):
                                # start_tensor_calc zeroes the whole PSUM
                                # bank; all 4 query-sub regions share one
                                # bank, so only the very first matmul starts
                                nc.tensor.matmul(
                                    ps_ctx[:, s4, :],
                                    ex[0:m, c - c0, 128 * s4:128 * (s4 + 1)],
                                    v_all[0:m, c, j, :],
                                    start=(c == 0 and s4 == 0),
                                    stop=(c == KC - 1),
                                    skip_group_check=True)
                        # interleave next qblock's Q projection mid-stream
                        if gi == 2 and j == 0 and qb + 1 < QC:
                            ps_q = ps_s.tile([128, EXP_GROUP, 512], F32,
                                             tag="s", name=f"psq{qb + 1}")
                            for c in range(8):
                                nc.tensor.matmul(
                                    ps_q[:, 0, :], wq_sb[:, c, :],
                                    qn[:, c, :], start=(c == 0), stop=(c == 7))
                            nc.vector.tensor_scalar_add(
                                q_pair[:, qb + 1, :], ps_q[:, 0, :], bq_sb[:])

                    if debug and j == 0 and qb == 0:
                        numcp = c_misc.tile([128, 4, DH + 1], F32, tag="ncp")
                        nc.vector.tensor_copy(numcp[:], ps_ctx[:])
                        nc.sync.dma_start(dbg_num[:], numcp[:])
                    # normalize: per-(query, sub) reciprocal of denominators
                    recip = c_misc.tile([128, 4, 1], F32, tag="r")
                    nc.vector.reciprocal(recip[:], ps_ctx[:, :, DH:DH + 1])
                    ctx_sb = c_misc.tile([128, 4, DH], BF16, tag="cs")
                    for s4 in range(4):
                        nc.vector.tensor_scalar_mul(
                            ctx_sb[:, s4, :], ps_ctx[:, s4, 0:DH],
                            recip[:, s4, :])
                    # transpose back to [64 f, 512 q] for the A2A payload
                    ps_t = ps_aux.tile([64, 512], BF16, tag="t")
                    for s4 in range(4):
                        nc.tensor.matmul(ps_t[:, 128 * s4:128 * (s4 + 1)],
                                         ctx_sb[:, s4, :], id_sb[:],
                                         is_transpose=True)
                    stage = c_misc.tile([64, 512], BF16, tag="st")
                    nc.vector.tensor_copy(stage[:], ps_t[:])
                    nc.gpsimd.dma_start(a2a_in[j][qb], stage[:])
                    if debug and j == 0 and qb == 0:
                        nc.sync.dma_start(dbg_ctx[:], ctx_sb[:])
                        nc.sync.dma_start(dbg_st[:], stage[:])

                if debug and j == HPC - 1:
                    nc.sync.dma_start(dbg_kT[:], kT_all[:])
                    nc.sync.dma_start(dbg_v[:], v_all[:])
                    nc.sync.dma_start(dbg_q[:], q_pair[:])
                nc.gpsimd.collective_compute(
                    "AllToAll", mybir.AluOpType.bypass,
                    replica_groups=[list(range(N_CORES))],
                    ins=[a2a_in[j].opt()],
                    outs=[a2a_out[j].opt()])

            # ---- phase D: output projection of the core's 512 rows ----
            ctx_p = [persist.tile([128, 4, 512], BF16, name=f"cxp{j}")
                     for j in range(HPC)]
            for j in range(HPC):
                ev = a2a_out[j].rearrange("(a two) p q -> a two p q", two=2)
                nc.sync.dma_start(ctx_p[j][0:64, :, :],
                                  ev[:, 0].rearrange("a p q -> p a q"))
                nc.sync.dma_start(ctx_p[j][64:128, :, :],
                                  ev[:, 1].rearrange("a p q -> p a q"))
                for qc in range(SQ // 128):
                    for eh in range(2):
                        es = slice(eh * 512, (eh + 1) * 512)
                        if j == 0:
                            ps_o = ps_aux.tile([128, 512], F32, tag="t",
                                               name=f"pso0_{qc}_{eh}")
                        else:
                            ps_o = ps_s.tile([128, EXP_GROUP, 512], F32,
                                             tag="s", name=f"pso1_{qc}_{eh}")
                            ps_o = ps_o[:, 0, :]
                        for a in range(4):
                            nc.tensor.matmul(
                                ps_o[:],
                                ctx_p[j][:, a, 128 * qc:128 * (qc + 1)],
                                wo_sb[:, 4 * j + a, es],
                                start=(a == 0),
                                stop=(j == 1 and a == 3))
                        if j == 0:
                            nc.tensor.matmul(ps_o[:], ones_bf[:, 0:128],
                                             bo_sb[:, es], start=False,
                                             stop=True)
                            nc.vector.tensor_copy(o_acc[:, qc, es], ps_o[:])
                        else:
                            o_sb = c_misc.tile([128, 512], F32, tag="osb")
                            nc.vector.tensor_add(o_sb[:], o_acc[:, qc, es],
                                                 ps_o[:])
                            nc.gpsimd.dma_start(
                                out[128 * qc:128 * (qc + 1), es], o_sb[:])

    nc.compile()
    return nc


def prepare(query, key, value, mask, Wq, bq, Wk, bk, Wv, bv, Wo, bo):
    """Host-side sharding/preprocessing + program build."""
    query = np.asarray(query)
    key = np.asarray(key)
    value = np.asarray(value)
    mask = np.asarray(mask)
    Wq, bq = np.asarray(Wq), np.asarray(bq)
    Wk, bk = np.asarray(Wk), np.asarray(bk)
    Wv, bv = np.asarray(Wv), np.asarray(bv)
    Wo, bo = np.asarray(Wo), np.asarray(bo)

    idx = np.nonzero(mask.reshape(-1) != 0)[0]
    n = int(idx.size)

    def pmajor(xT):
        # [1024, cols] feature-major -> [128, 8, cols] partition-major
        return np.ascontiguousarray(
            xT.reshape(8, 128, xT.shape[1]).transpose(1, 0, 2))

    q_p = pmajor(_bf16(query[0].T))
    kc_p = pmajor(_bf16(key[0, idx, :].T))
    vc_p = pmajor(_bf16(value[0, idx, :].T))

    wqT = _bf16(Wq.T)   # [1024 in, 1024 out]
    wkT = _bf16(Wk.T)
    wvT = _bf16(Wv.T)
    woT_r = Wo.T
    slots = []
    for j in range(HPC):
        for a in range(4):
            hA, hB = 4 * a + j, 4 * a + 2 + j
            slots.append(woT_r[64 * hA:64 * hA + 64, :])
            slots.append(woT_r[64 * hB:64 * hB + 64, :])
    wo_p = pmajor(_bf16(np.concatenate(slots, axis=0)))
    bo_r = _bf16(bo.reshape(1, D))
    ident = np.eye(128, dtype=ml_dtypes.bfloat16)

    nc = build_program(n)

    in_maps = []
    for m in range(N_CORES):
        sl = slice(m * 128, (m + 1) * 128)
        in_maps.append({
            "q_p": q_p,
            "kc_p": kc_p,
            "vc_p": vc_p,
            "wq_p": pmajor(np.ascontiguousarray(wqT[:, sl])),
            "wk_p": pmajor(np.ascontiguousarray(wkT[:, sl])),
            "wv_p": pmajor(np.ascontiguousarray(wvT[:, sl])),
            "wo_p": wo_p,
            "bq_m": np.ascontiguousarray(
                bq[sl].reshape(128, 1).astype(np.float32)),
            "bk_m": np.ascontiguousarray(
                bk[sl].reshape(128, 1).astype(np.float32)),
            "bv_r": _bf16(bv[sl].reshape(1, 128)),
            "bo_r": bo_r,
            "ident": ident,
        })

    return {"nc": nc, "in_maps": in_maps, "n": n}


def kernel(query, key, value, mask, Wq, bq, Wk, bk, Wv, bv, Wo, bo,
           _trace=False, _result_box=None):
    prep = prepare(query, key, value, mask, Wq, bq, Wk, bk, Wv, bv, Wo, bo)
    res = run_bass_kernel_spmd(prep["nc"], prep["in_maps"],
                               list(range(N_CORES)), trace=_trace)
    if _result_box is not None:
        _result_box.append(res)

    out = np.concatenate([res.results[m]["out"] for m in range(N_CORES)],
                         axis=0)
    return out.reshape(1, S, D).astype(np.float32)


# revision 12
# speedup vs baseline: 1.3628x; 1.0628x over previous
"""Multi-head attention (B=1, S=4096, D=1024, H=16) on 8 TRN2 NeuronCores.

Strategy (head-sharded attention + AllToAll context exchange), v2:
  - Host: compact K/V to the unmasked key positions (mask==0 keys contribute
    exactly 0 to softmax numerator and denominator since exp underflows),
    re-layout activations/weights partition-major, cast matmul operands bf16.
    No padding of the key count: the last 128-chunk is partial (M=44).
  - Core m owns heads 2m, 2m+1.  K projection -> kT_all [128(2h x 64dh), n];
    V projection -> v_all [128 keys, chunk, head, 65] (col 64 = ones, so the
    PV matmul also produces softmax denominators).
  - Attention per (head, qblock of 512 queries): scores^T chunks [128k, 512q]
    in PSUM groups of 3 banks, exp on ScalarE (scale=1/8), then PV with the
    exp tile as the *stationary* operand: out ctx [128 q, 65] per 128-query
    sub-block -- the moving dim is only 65 wide, which the PE cost model
    (cycles ~ moving size) makes ~2x cheaper than the [65, 512] orientation.
  - Normalize with per-partition reciprocal (DVE tensor_scalar), transpose
    ctx back to [64f, 512q] on the PE (identity matmul), stage and DMA to the
    per-head AllToAll buffer.  One AllToAll per head; the first overlaps the
    second head's compute.
  - Phase D: output projection of the core's own 512 query rows (query-
    sharded after the AllToAll), bias via ones-row matmul.
"""

import numpy as np
import ml_dtypes

import concourse.bacc as bacc
import concourse.mybir as mybir
import concourse.tile as tile
from concourse.bass_utils import run_bass_kernel_spmd

HEADS = 16
D = 1024
DH = 64
S = 4096
N_CORES = 8
SQ = S // N_CORES          # query rows owned per core (output sharding)
HPC = HEADS // N_CORES     # heads per core
QC = S // 512              # 512-query blocks over the whole sequence
BF16 = mybir.dt.bfloat16
F32 = mybir.dt.float32
EXP_GROUP = 3              # k-chunks (PSUM banks) per exp activation op
KEEPALIVE = 90             # PE keep-alive matmuls spanning the 2nd AllToAll


def _bf16(x):
    return np.ascontiguousarray(np.asarray(x).astype(ml_dtypes.bfloat16))


def build_program(n):
    """Build the 8-core SPMD program for n (unpadded) compacted keys."""
    KC = (n + 127) // 128                    # 128-key chunks, last partial
    groups = [(c0, min(c0 + EXP_GROUP, KC)) for c0 in range(0, KC, EXP_GROUP)]
    mc = [min(128, n - 128 * c) for c in range(KC)]   # keys in chunk c
    # key-load column groups of <=512 for the projection pipeline
    kgs = [(g0, min(g0 + 512, n)) for g0 in range(0, n, 512)]

    nc = bacc.Bacc("TRN2", target_bir_lowering=False, debug=False,
                   num_devices=N_CORES)

    # ---- I/O ----  (partition-major [128, 8, cols] layouts, bf16)
    q_p = nc.dram_tensor("q_p", [128, 8, S], BF16, kind="ExternalInput")
    kc_p = nc.dram_tensor("kc_p", [128, 8, n], BF16, kind="ExternalInput")
    vc_p = nc.dram_tensor("vc_p", [128, 8, n], BF16, kind="ExternalInput")
    wq_p = nc.dram_tensor("wq_p", [128, 8, 128], BF16, kind="ExternalInput")
    wk_p = nc.dram_tensor("wk_p", [128, 8, 128], BF16, kind="ExternalInput")
    wv_p = nc.dram_tensor("wv_p", [128, 8, 128], BF16, kind="ExternalInput")
    wo_p = nc.dram_tensor("wo_p", [128, 8, D], BF16, kind="ExternalInput")
    bq_m = nc.dram_tensor("bq_m", [128, 1], F32, kind="ExternalInput")
    bk_m = nc.dram_tensor("bk_m", [128, 1], F32, kind="ExternalInput")
    bv_r = nc.dram_tensor("bv_r", [1, 128], BF16, kind="ExternalInput")
    bo_r = nc.dram_tensor("bo_r", [1, D], BF16, kind="ExternalInput")
    ident = nc.dram_tensor("ident", [128, 128], BF16, kind="ExternalInput")
    out = nc.dram_tensor("out", [SQ, D], F32, kind="ExternalOutput")

    with tile.TileContext(nc) as tc:
        with (
            tc.tile_pool(name="dram", bufs=1, space="DRAM") as dram,
            tc.tile_pool(name="consts", bufs=1) as consts,
            tc.tile_pool(name="persist", bufs=1) as persist,
            tc.tile_pool(name="qld", bufs=2) as qld,
            tc.tile_pool(name="c_exp", bufs=3) as c_exp,
            tc.tile_pool(name="c_misc", bufs=2) as c_misc,
            tc.tile_pool(name="ps_s", bufs=2, space="PSUM") as ps_s,
            tc.tile_pool(name="ps_aux", bufs=1, space="PSUM") as ps_aux,
        ):
            # per-head A2A buffers: dest qblock -> [64 feats, 512 q]
            a2a_in = [dram.tile([N_CORES, 64, 512], BF16, name=f"a2i{j}")
                      for j in range(HPC)]
            a2a_out = [dram.tile([N_CORES, 64, 512], BF16, name=f"a2o{j}")
                       for j in range(HPC)]

            # ---- consts / weights ----
            wq_sb = consts.tile([128, 8, 128], BF16)
            nc.sync.dma_start(wq_sb[:], wq_p[:])
            wk_sb = consts.tile([128, 8, 128], BF16)
            nc.sync.dma_start(wk_sb[:], wk_p[:])
            wv_sb = consts.tile([128, 8, 128], BF16)
            nc.sync.dma_start(wv_sb[:], wv_p[:])
            bq_sb = consts.tile([128, 1], F32)
            nc.sync.dma_start(bq_sb[:], bq_m[:])
            bk_sb = consts.tile([128, 1], F32)
            nc.sync.dma_start(bk_sb[:], bk_m[:])
            bv_sb = consts.tile([1, 128], BF16)
            nc.sync.dma_start(bv_sb[:], bv_r[:])
            bo_sb = consts.tile([1, D], BF16)
            nc.sync.dma_start(bo_sb[:], bo_r[:])
            id_sb = consts.tile([128, 128], BF16)
            nc.sync.dma_start(id_sb[:], ident[:])
            ones_bf = consts.tile([1, 128], BF16)
            nc.vector.memset(ones_bf[:], 1.0)

            # ---- persistent state ----
            kT_all = persist.tile([128, n], BF16)
            v_all = persist.tile([128, KC, HPC, DH + 1], BF16)
            q_pair = persist.tile([128, QC, 512], BF16)
            kin = persist.tile([128, 8, n], BF16)
            vin = persist.tile([128, 8, n], BF16)
            wo_sb = persist.tile([128, 8, D], BF16)
            o_acc = persist.tile([128, SQ // 128, D], F32)
            if mc[-1] < 128:
                # partial last chunk: the PE contraction tile rounds up past
                # the real key count, so the tail rows must multiply to zero
                nc.vector.memset(v_all[:, KC - 1, :, :], 0.0)
                nc.vector.memset(v_all[:, 0:KC - 1, :, DH:DH + 1], 1.0)
                nc.vector.memset(v_all[0:mc[-1], KC - 1, :, DH:DH + 1], 1.0)
            else:
                nc.vector.memset(v_all[:, :, :, DH:DH + 1], 1.0)

            # ---- input loads (SP queue; staggered for early start) ----
            q0 = qld.tile([128, 8, 512], BF16, name="qt0", tag="q")
            g0, g1 = kgs[0]
            nc.sync.dma_start(kin[:, :, g0:g1], kc_p[:, :, g0:g1])
            nc.sync.dma_start(q0[:], q_p[:, :, 0:512])
            nc.sync.dma_start(vin[:, :, g0:g1], vc_p[:, :, g0:g1])
            for (g0, g1) in kgs[1:]:
                nc.sync.dma_start(kin[:, :, g0:g1], kc_p[:, :, g0:g1])
                nc.sync.dma_start(vin[:, :, g0:g1], vc_p[:, :, g0:g1])
            nc.scalar.dma_start(wo_sb[:], wo_p[:])   # used only in phase D

            # ---- phase A, emitted lazily inside the first qblock ----
            a_kg = [0]     # next K-projection load-group to emit
            a_vc = [0]     # next V-projection chunk to emit

            def emit_k_group():
                g0, g1 = kgs[a_kg[0]]
                a_kg[0] += 1
                kn = g1 - g0
                ps_k = ps_s.tile([128, EXP_GROUP, 512], F32, tag="s",
                                 name=f"psk{g0}")
                for c in range(8):
                    nc.tensor.matmul(ps_k[:, 0, 0:kn], wk_sb[:, c, :],
                                     kin[:, c, g0:g1],
                                     start=(c == 0), stop=(c == 7))
                nc.vector.tensor_scalar_add(kT_all[:, g0:g1],
                                            ps_k[:, 0, 0:kn], bk_sb[:])

            def ensure_k(chunks):
                while a_kg[0] * 4 < chunks and a_kg[0] < len(kgs):
                    emit_k_group()

            def ensure_v(chunks):
                while a_vc[0] < min(chunks, KC):
                    c = a_vc[0]
                    a_vc[0] += 1
                    m = mc[c]
                    ks = slice(128 * c, 128 * c + m)
                    ps_v = ps_s.tile([128, EXP_GROUP, 512], F32, tag="s",
                                     name=f"psv{c}")
                    pv = ps_v[0:m, 0, 0:128]
                    for cc in range(8):
                        nc.tensor.matmul(pv, vin[:, cc, ks], wv_sb[:, cc, :],
                                         start=(cc == 0), stop=False)
                    nc.tensor.matmul(pv, ones_bf[:, 0:m], bv_sb[:],
                                     start=False, stop=True)
                    nc.vector.tensor_copy(
                        v_all[0:m, c, :, 0:DH],
                        pv.rearrange("p (j f) -> p j f", j=HPC))

            def emit_qproj(qb, qtile):
                ps_q = ps_s.tile([128, EXP_GROUP, 512], F32, tag="s",
                                 name=f"psq{qb}")
                for c in range(8):
                    nc.tensor.matmul(ps_q[:, 0, :], wq_sb[:, c, :],
                                     qtile[:, c, :],
                                     start=(c == 0), stop=(c == 7))
                nc.vector.tensor_scalar_add(q_pair[:, qb, :], ps_q[:, 0, :],
                                            bq_sb[:])

            # ---- deferred per-(head, qblock) finalize: normalize the PV
            # accumulator, transpose, stage and ship to the A2A buffer.
            # Emitted one iteration late so the next qblock's first score
            # group is already in the PE stream (no ScalarE bubble).
            def finalize(j, qb, ps_ctx):
                recip = c_misc.tile([128, 4, 1], F32, tag="r",
                                    name=f"rc{j}_{qb}")
                nc.vector.reciprocal(recip[:], ps_ctx[:, :, DH:DH + 1])
                ctx_sb = c_misc.tile([128, 4, DH], BF16, tag="cs",
                                     name=f"cs{j}_{qb}")
                for s4 in range(4):
                    nc.vector.tensor_scalar_mul(
                        ctx_sb[:, s4, :], ps_ctx[:, s4, 0:DH],
                        recip[:, s4, :])
                ps_t = ps_aux.tile([64, 512], BF16, tag="t",
                                   name=f"pt{j}_{qb}")
                for s4 in range(4):
                    nc.tensor.matmul(ps_t[:, 128 * s4:128 * (s4 + 1)],
                                     ctx_sb[:, s4, :], id_sb[:],
                                     is_transpose=True)
                stage = c_misc.tile([64, 512], BF16, tag="st",
                                    name=f"sg{j}_{qb}")
                nc.vector.tensor_copy(stage[:], ps_t[:])
                nc.gpsimd.dma_start(a2a_in[j][qb], stage[:])
                if qb == QC - 1:
                    nc.gpsimd.collective_compute(
                        "AllToAll", mybir.AluOpType.bypass,
                        replica_groups=[list(range(N_CORES))],
                        ins=[a2a_in[j].opt()],
                        outs=[a2a_out[j].opt()])

            # ---- phase C: attention (head-major; qblock inner) ----
            pend = None
            for j in range(HPC):
                pj = slice(64 * j, 64 * (j + 1))
                for qb in range(QC):
                    first = (j == 0 and qb == 0)
                    if j == 0 and qb + 1 < QC:
                        qn = qld.tile([128, 8, 512], BF16, name=f"qt{qb + 1}",
                                      tag="q")
                        nc.sync.dma_start(
                            qn[:], q_p[:, :, 512 * (qb + 1):512 * (qb + 2)])
                    ps_ctx = ps_aux.tile([128, 4, DH + 1], F32, tag="ctx",
                                         name=f"pc{j}_{qb}")
                    for gi, (c0, c1) in enumerate(groups):
                        gn = c1 - c0
                        if first:
                            ensure_k(c1)
                            if gi == 0:
                                emit_qproj(0, q0)
                        ps = ps_s.tile([128, EXP_GROUP, 512], F32, tag="s")
                        for c in range(c0, c1):
                            m = mc[c]
                            nc.tensor.matmul(
                                ps[0:m, c - c0, :],
                                kT_all[pj, 128 * c:128 * c + m],
                                q_pair[pj, qb, :], start=True, stop=True,
                                tile_position=(64 * j, 0))
                        ex = c_exp.tile([128, EXP_GROUP, 512], BF16, tag="e")
                        nc.scalar.activation(
                            ex[:, 0:gn, :], ps[:, 0:gn, :],
                            mybir.ActivationFunctionType.Exp,
                            bias=0.0, scale=0.125)
                        if gi == 0 and pend is not None:
                            finalize(*pend)
                            pend = None
                        if first:
                            ensure_v(c1)
                        for c in range(c0, c1):
                            m = mc[c]
                            for s4 in range(4):
                                # start_tensor_calc zeroes the whole PSUM
                                # bank; all 4 query-sub regions share one
                                # bank, so only the very first matmul starts
                                nc.tensor.matmul(
                                    ps_ctx[:, s4, :],
                                    ex[0:m, c - c0, 128 * s4:128 * (s4 + 1)],
                                    v_all[0:m, c, j, :],
                                    start=(c == 0 and s4 == 0),
                                    stop=(c == KC - 1),
                                    skip_group_check=True)
                        # interleave next qblock's Q projection mid-stream
                        if gi == 2 and j == 0 and qb + 1 < QC:
                            emit_qproj(qb + 1, qn)
                    pend = (j, qb, ps_ctx)
            finalize(*pend)   # last qblock + second AllToAll

            # ---- phase D: output projection of the core's 512 rows ----
            ctx_p = [persist.tile([128, 4, 512], BF16, name=f"cxp{j}")
                     for j in range(HPC)]
            for j in range(HPC):
                ev = a2a_out[j].rearrange("(a two) p q -> a two p q", two=2)
                nc.sync.dma_start(ctx_p[j][0:64, :, :],
                                  ev[:, 0].rearrange("a p q -> p a q"))
                nc.sync.dma_start(ctx_p[j][64:128, :, :],
                                  ev[:, 1].rearrange("a p q -> p a q"))
                for qc in range(SQ // 128):
                    for eh in range(2):
                        es = slice(eh * 512, (eh + 1) * 512)
                        if j == 0:
                            ps_o = ps_aux.tile([128, 512], F32, tag="t",
                                               name=f"pso0_{qc}_{eh}")
                        else:
                            ps_o = ps_s.tile([128, EXP_GROUP, 512], F32,
                                             tag="s", name=f"pso1_{qc}_{eh}")
                            ps_o = ps_o[:, 0, :]
                        for a in range(4):
                            nc.tensor.matmul(
                                ps_o[:],
                                ctx_p[j][:, a, 128 * qc:128 * (qc + 1)],
                                wo_sb[:, 4 * j + a, es],
                                start=(a == 0),
                                stop=(j == 1 and a == 3))
                        if j == 0:
                            nc.tensor.matmul(ps_o[:], ones_bf[:, 0:128],
                                             bo_sb[:, es], start=False,
                                             stop=True)
                            nc.vector.tensor_copy(o_acc[:, qc, es], ps_o[:])
                        else:
                            o_sb = c_misc.tile([128, 512], F32, tag="osb")
                            nc.vector.tensor_add(o_sb[:], o_acc[:, qc, es],
                                                 ps_o[:])
                            nc.sync.dma_start(
                                out[128 * qc:128 * (qc + 1), es], o_sb[:])
                if j == 0:
                    # keep the PE clock ramped through the second AllToAll:
                    # idle gaps reset the p-state and would double the cost
                    # of the j=1 output projection on the tail
                    ka = ps_aux.tile([128, 512], F32, tag="t", name="ka")
                    for i in range(KEEPALIVE):
                        nc.tensor.matmul(ka[:], kT_all[0:64, 0:128],
                                         kT_all[0:64, 0:512],
                                         start=True, stop=True)

    nc.compile()
    return nc


def prepare(query, key, value, mask, Wq, bq, Wk, bk, Wv, bv, Wo, bo):
    """Host-side sharding/preprocessing + program build."""
    query = np.asarray(query)
    key = np.asarray(key)
    value = np.asarray(value)
    mask = np.asarray(mask)
    Wq, bq = np.asarray(Wq), np.asarray(bq)
    Wk, bk = np.asarray(Wk), np.asarray(bk)
    Wv, bv = np.asarray(Wv), np.asarray(bv)
    Wo, bo = np.asarray(Wo), np.asarray(bo)

    idx = np.nonzero(mask.reshape(-1) != 0)[0]
    n = int(idx.size)

    def pmajor(xT):
        # [1024, cols] feature-major -> [128, 8, cols] partition-major
        return np.ascontiguousarray(
            xT.reshape(8, 128, xT.shape[1]).transpose(1, 0, 2))

    q_p = pmajor(_bf16(query[0].T))
    kc_p = pmajor(_bf16(key[0, idx, :].T))
    vc_p = pmajor(_bf16(value[0, idx, :].T))

    wqT = _bf16(Wq.T)   # [1024 in, 1024 out]
    wkT = _bf16(Wk.T)
    wvT = _bf16(Wv.T)
    woT_r = Wo.T
    slots = []
    for j in range(HPC):
        for a in range(4):
            hA, hB = 4 * a + j, 4 * a + 2 + j
            slots.append(woT_r[64 * hA:64 * hA + 64, :])
            slots.append(woT_r[64 * hB:64 * hB + 64, :])
    wo_p = pmajor(_bf16(np.concatenate(slots, axis=0)))
    bo_r = _bf16(bo.reshape(1, D))
    ident = np.eye(128, dtype=ml_dtypes.bfloat16)

    nc = build_program(n)

    in_maps = []
    for m in range(N_CORES):
        sl = slice(m * 128, (m + 1) * 128)
        in_maps.append({
            "q_p": q_p,
            "kc_p": kc_p,
            "vc_p": vc_p,
            "wq_p": pmajor(np.ascontiguousarray(wqT[:, sl])),
            "wk_p": pmajor(np.ascontiguousarray(wkT[:, sl])),
            "wv_p": pmajor(np.ascontiguousarray(wvT[:, sl])),
            "wo_p": wo_p,
            "bq_m": np.ascontiguousarray(
                bq[sl].reshape(128, 1).astype(np.float32)),
            "bk_m": np.ascontiguousarray(
                bk[sl].reshape(128, 1).astype(np.float32)),
            "bv_r": _bf16(bv[sl].reshape(1, 128)),
            "bo_r": bo_r,
            "ident": ident,
        })

    return {"nc": nc, "in_maps": in_maps, "n": n}


def kernel(query, key, value, mask, Wq, bq, Wk, bk, Wv, bv, Wo, bo,
           _trace=False, _result_box=None):
    prep = prepare(query, key, value, mask, Wq, bq, Wk, bk, Wv, bv, Wo, bo)
    res = run_bass_kernel_spmd(prep["nc"], prep["in_maps"],
                               list(range(N_CORES)), trace=_trace)
    if _result_box is not None:
        _result_box.append(res)

    out = np.concatenate([res.results[m]["out"] for m in range(N_CORES)],
                         axis=0)
    return out.reshape(1, S, D).astype(np.float32)


# revision 13
# speedup vs baseline: 1.3628x; 1.0000x over previous
"""Multi-head attention (B=1, S=4096, D=1024, H=16) on 8 TRN2 NeuronCores.

Strategy (head-sharded attention + AllToAll context exchange), v2:
  - Host: compact K/V to the unmasked key positions (mask==0 keys contribute
    exactly 0 to softmax numerator and denominator since exp underflows),
    re-layout activations/weights partition-major, cast matmul operands bf16.
    No padding of the key count: the last 128-chunk is partial (M=44).
  - Core m owns heads 2m, 2m+1.  K projection -> kT_all [128(2h x 64dh), n];
    V projection -> v_all [128 keys, chunk, head, 65] (col 64 = ones, so the
    PV matmul also produces softmax denominators).
  - Attention per (head, qblock of 512 queries): scores^T chunks [128k, 512q]
    in PSUM groups of 3 banks, exp on ScalarE (scale=1/8), then PV with the
    exp tile as the *stationary* operand: out ctx [128 q, 65] per 128-query
    sub-block -- the moving dim is only 65 wide, which the PE cost model
    (cycles ~ moving size) makes ~2x cheaper than the [65, 512] orientation.
  - Normalize with per-partition reciprocal (DVE tensor_scalar), transpose
    ctx back to [64f, 512q] on the PE (identity matmul), stage and DMA to the
    per-head AllToAll buffer.  One AllToAll per head; the first overlaps the
    second head's compute.
  - Phase D: output projection of the core's own 512 query rows (query-
    sharded after the AllToAll), bias via ones-row matmul.
"""

import numpy as np
import ml_dtypes

import concourse.bacc as bacc
import concourse.mybir as mybir
import concourse.tile as tile
from concourse.bass_utils import run_bass_kernel_spmd

HEADS = 16
D = 1024
DH = 64
S = 4096
N_CORES = 8
SQ = S // N_CORES          # query rows owned per core (output sharding)
HPC = HEADS // N_CORES     # heads per core
QC = S // 512              # 512-query blocks over the whole sequence
BF16 = mybir.dt.bfloat16
F32 = mybir.dt.float32
EXP_GROUP = 3              # k-chunks (PSUM banks) per exp activation op
KEEPALIVE = 110            # PE keep-alive matmuls spanning the 2nd AllToAll
START_KEEPALIVE = 24       # PE warm-up matmuls at program start


def _bf16(x):
    return np.ascontiguousarray(np.asarray(x).astype(ml_dtypes.bfloat16))


def build_program(n):
    """Build the 8-core SPMD program for n (unpadded) compacted keys."""
    KC = (n + 127) // 128                    # 128-key chunks, last partial
    groups = [(c0, min(c0 + EXP_GROUP, KC)) for c0 in range(0, KC, EXP_GROUP)]
    mc = [min(128, n - 128 * c) for c in range(KC)]   # keys in chunk c
    # key-load column groups of 3 chunks, aligned with the score groups
    kgs = [(g0, min(g0 + 384, n)) for g0 in range(0, n, 384)]

    nc = bacc.Bacc("TRN2", target_bir_lowering=False, debug=False,
                   num_devices=N_CORES)

    # ---- I/O ----  (partition-major [128, 8, cols] layouts, bf16)
    q_p = nc.dram_tensor("q_p", [128, 8, S], BF16, kind="ExternalInput")
    kc_p = nc.dram_tensor("kc_p", [128, 8, n], BF16, kind="ExternalInput")
    vc_p = nc.dram_tensor("vc_p", [128, 8, n], BF16, kind="ExternalInput")
    wq_p = nc.dram_tensor("wq_p", [128, 8, 128], BF16, kind="ExternalInput")
    wk_p = nc.dram_tensor("wk_p", [128, 8, 128], BF16, kind="ExternalInput")
    wv_p = nc.dram_tensor("wv_p", [128, 8, 128], BF16, kind="ExternalInput")
    wo_p = nc.dram_tensor("wo_p", [128, 8, D], BF16, kind="ExternalInput")
    bq_m = nc.dram_tensor("bq_m", [128, 1], F32, kind="ExternalInput")
    bk_m = nc.dram_tensor("bk_m", [128, 1], F32, kind="ExternalInput")
    bv_r = nc.dram_tensor("bv_r", [1, 128], BF16, kind="ExternalInput")
    bo_r = nc.dram_tensor("bo_r", [1, D], BF16, kind="ExternalInput")
    ident = nc.dram_tensor("ident", [128, 128], BF16, kind="ExternalInput")
    out = nc.dram_tensor("out", [SQ, D], F32, kind="ExternalOutput")

    with tile.TileContext(nc) as tc:
        with (
            tc.tile_pool(name="dram", bufs=1, space="DRAM") as dram,
            tc.tile_pool(name="consts", bufs=1) as consts,
            tc.tile_pool(name="persist", bufs=1) as persist,
            tc.tile_pool(name="qld", bufs=3) as qld,
            tc.tile_pool(name="c_exp", bufs=3) as c_exp,
            tc.tile_pool(name="c_misc", bufs=2) as c_misc,
            tc.tile_pool(name="ps_s", bufs=2, space="PSUM") as ps_s,
            tc.tile_pool(name="ps_aux", bufs=1, space="PSUM") as ps_aux,
        ):
            # per-head A2A buffers: dest qblock -> [64 feats, 512 q]
            a2a_in = [dram.tile([N_CORES, 64, 512], BF16, name=f"a2i{j}")
                      for j in range(HPC)]
            a2a_out = [dram.tile([N_CORES, 64, 512], BF16, name=f"a2o{j}")
                       for j in range(HPC)]

            # ---- consts / weights ----
            wq_sb = consts.tile([128, 8, 128], BF16)
            nc.sync.dma_start(wq_sb[:], wq_p[:])
            wk_sb = consts.tile([128, 8, 128], BF16)
            nc.sync.dma_start(wk_sb[:], wk_p[:])
            wv_sb = consts.tile([128, 8, 128], BF16)
            nc.sync.dma_start(wv_sb[:], wv_p[:])
            bq_sb = consts.tile([128, 1], F32)
            nc.sync.dma_start(bq_sb[:], bq_m[:])
            bk_sb = consts.tile([128, 1], F32)
            nc.sync.dma_start(bk_sb[:], bk_m[:])
            bv_sb = consts.tile([1, 128], BF16)
            nc.sync.dma_start(bv_sb[:], bv_r[:])
            bo_sb = consts.tile([1, D], BF16)
            nc.sync.dma_start(bo_sb[:], bo_r[:])
            id_sb = consts.tile([128, 128], BF16)
            nc.sync.dma_start(id_sb[:], ident[:])
            ones_bf = consts.tile([1, 512], BF16)
            nc.vector.memset(ones_bf[:], 1.0)
            # warm the PE clock ramp while the first loads are in flight
            ka0 = ps_aux.tile([128, 512], F32, tag="t", name="ka0")
            for _ in range(START_KEEPALIVE):
                nc.tensor.matmul(ka0[:], ones_bf[:, 0:128], ones_bf[:],
                                 start=True, stop=True)

            # ---- persistent state ----
            kT_all = persist.tile([128, n], BF16)
            v_all = persist.tile([128, KC, HPC, DH + 1], BF16)
            q_pair = persist.tile([128, QC, 512], BF16)
            kin = persist.tile([128, 8, n], BF16)
            vin = persist.tile([128, 8, n], BF16)
            wo_sb = persist.tile([128, 8, D], BF16)
            o_acc = persist.tile([128, SQ // 128, D], F32)
            if mc[-1] < 128:
                # partial last chunk: the PE contraction tile rounds up past
                # the real key count, so the tail rows must multiply to zero
                nc.vector.memset(v_all[:, KC - 1, :, :], 0.0)
                nc.vector.memset(v_all[:, 0:KC - 1, :, DH:DH + 1], 1.0)
                nc.vector.memset(v_all[0:mc[-1], KC - 1, :, DH:DH + 1], 1.0)
            else:
                nc.vector.memset(v_all[:, :, :, DH:DH + 1], 1.0)

            # ---- input loads (SP queue; staggered for early start) ----
            q0 = qld.tile([128, 8, 512], BF16, name="qt0", tag="q")
            q1 = qld.tile([128, 8, 512], BF16, name="qt1", tag="q")
            g0, g1 = kgs[0]
            nc.sync.dma_start(kin[:, :, g0:g1], kc_p[:, :, g0:g1])
            nc.sync.dma_start(q0[:], q_p[:, :, 0:512])
            nc.sync.dma_start(vin[:, :, g0:g1], vc_p[:, :, g0:g1])
            for i, (g0, g1) in enumerate(kgs[1:]):
                nc.sync.dma_start(kin[:, :, g0:g1], kc_p[:, :, g0:g1])
                if i == 0:
                    nc.sync.dma_start(q1[:], q_p[:, :, 512:1024])
                nc.sync.dma_start(vin[:, :, g0:g1], vc_p[:, :, g0:g1])
            nc.scalar.dma_start(wo_sb[:], wo_p[:])   # used only in phase D

            # ---- phase A, emitted lazily inside the first qblock ----
            a_kg = [0]     # next K-projection load-group to emit
            a_vc = [0]     # next V-projection chunk to emit

            def emit_k_group():
                g0, g1 = kgs[a_kg[0]]
                a_kg[0] += 1
                kn = g1 - g0
                ps_k = ps_s.tile([128, EXP_GROUP, 512], F32, tag="s",
                                 name=f"psk{g0}")
                for c in range(8):
                    nc.tensor.matmul(ps_k[:, 0, 0:kn], wk_sb[:, c, :],
                                     kin[:, c, g0:g1],
                                     start=(c == 0), stop=(c == 7))
                nc.vector.tensor_scalar_add(kT_all[:, g0:g1],
                                            ps_k[:, 0, 0:kn], bk_sb[:])

            def ensure_k(chunks):
                while a_kg[0] * 3 < chunks and a_kg[0] < len(kgs):
                    emit_k_group()

            def ensure_v(chunks):
                while a_vc[0] < min(chunks, KC):
                    c = a_vc[0]
                    a_vc[0] += 1
                    m = mc[c]
                    ks = slice(128 * c, 128 * c + m)
                    ps_v = ps_s.tile([128, EXP_GROUP, 512], F32, tag="s",
                                     name=f"psv{c}")
                    pv = ps_v[0:m, 0, 0:128]
                    for cc in range(8):
                        nc.tensor.matmul(pv, vin[:, cc, ks], wv_sb[:, cc, :],
                                         start=(cc == 0), stop=False)
                    nc.tensor.matmul(pv, ones_bf[:, 0:m], bv_sb[:],
                                     start=False, stop=True)
                    nc.vector.tensor_copy(
                        v_all[0:m, c, :, 0:DH],
                        pv.rearrange("p (j f) -> p j f", j=HPC))

            def emit_qproj(qb, qtile):
                ps_q = ps_s.tile([128, EXP_GROUP, 512], F32, tag="s",
                                 name=f"psq{qb}")
                for c in range(8):
                    nc.tensor.matmul(ps_q[:, 0, :], wq_sb[:, c, :],
                                     qtile[:, c, :],
                                     start=(c == 0), stop=(c == 7))
                nc.vector.tensor_scalar_add(q_pair[:, qb, :], ps_q[:, 0, :],
                                            bq_sb[:])

            # ---- deferred per-(head, qblock) finalize: normalize the PV
            # accumulator, transpose, stage and ship to the A2A buffer.
            # Emitted one iteration late so the next qblock's first score
            # group is already in the PE stream (no ScalarE bubble).
            def finalize(j, qb, ps_ctx, last_pv):
                emit_pv(*last_pv)
                recip = c_misc.tile([128, 4, 1], F32, tag="r",
                                    name=f"rc{j}_{qb}")
                nc.vector.reciprocal(recip[:], ps_ctx[:, :, DH:DH + 1])
                ctx_sb = c_misc.tile([128, 4, DH], BF16, tag="cs",
                                     name=f"cs{j}_{qb}")
                for s4 in range(4):
                    nc.vector.tensor_scalar_mul(
                        ctx_sb[:, s4, :], ps_ctx[:, s4, 0:DH],
                        recip[:, s4, :])
                ps_t = ps_aux.tile([64, 512], BF16, tag="t",
                                   name=f"pt{j}_{qb}")
                for s4 in range(4):
                    nc.tensor.matmul(ps_t[:, 128 * s4:128 * (s4 + 1)],
                                     ctx_sb[:, s4, :], id_sb[:],
                                     is_transpose=True)
                stage = c_misc.tile([64, 512], BF16, tag="st",
                                    name=f"sg{j}_{qb}")
                nc.vector.tensor_copy(stage[:], ps_t[:])
                nc.gpsimd.dma_start(a2a_in[j][qb], stage[:])
                if qb == QC - 1:
                    nc.gpsimd.collective_compute(
                        "AllToAll", mybir.AluOpType.bypass,
                        replica_groups=[list(range(N_CORES))],
                        ins=[a2a_in[j].opt()],
                        outs=[a2a_out[j].opt()])

            # ---- phase C: attention (head-major; qblock inner) ----
            qtiles = {0: q0, 1: q1}
            pend = None
            for j in range(HPC):
                pj = slice(64 * j, 64 * (j + 1))
                for qb in range(QC):
                    first = (j == 0 and qb == 0)
                    if j == 0 and qb + 2 < QC:
                        qt = qld.tile([128, 8, 512], BF16, name=f"qt{qb + 2}",
                                      tag="q")
                        nc.sync.dma_start(
                            qt[:], q_p[:, :, 512 * (qb + 2):512 * (qb + 3)])
                        qtiles[qb + 2] = qt
                    ps_ctx = ps_aux.tile([128, 4, DH + 1], F32, tag="ctx",
                                         name=f"pc{j}_{qb}")
                    for gi, (c0, c1) in enumerate(groups):
                        gn = c1 - c0
                        if first:
                            ensure_k(c1)
                            if gi == 0:
                                emit_qproj(0, q0)
                        ps = ps_s.tile([128, EXP_GROUP, 512], F32, tag="s")
                        for c in range(c0, c1):
                            m = mc[c]
                            nc.tensor.matmul(
                                ps[0:m, c - c0, :],
                                kT_all[pj, 128 * c:128 * c + m],
                                q_pair[pj, qb, :], start=True, stop=True,
                                tile_position=(64 * j, 0))
                        ex = c_exp.tile([128, EXP_GROUP, 512], BF16, tag="e")
                        nc.scalar.activation(
                            ex[:, 0:gn, :], ps[:, 0:gn, :],
                            mybir.ActivationFunctionType.Exp,
                            bias=0.0, scale=0.125)
                        if gi == 0 and pend is not None:
                            finalize(*pend)
                            pend = None
                        if first:
                            ensure_v(c1)

                        def emit_pv(ps_ctx, ex, c0, c1, j):
                            for c in range(c0, c1):
                                m = mc[c]
                                for s4 in range(4):
                                    # start_tensor_calc zeroes the whole
                                    # PSUM bank; all 4 query-sub regions
                                    # share one bank, so only the very
                                    # first matmul starts
                                    nc.tensor.matmul(
                                        ps_ctx[:, s4, :],
                                        ex[0:m, c - c0,
                                           128 * s4:128 * (s4 + 1)],
                                        v_all[0:m, c, j, :],
                                        start=(c == 0 and s4 == 0),
                                        stop=(c == KC - 1),
                                        skip_group_check=True)

                        if gi + 1 < len(groups):
                            emit_pv(ps_ctx, ex, c0, c1, j)
                        else:
                            # defer the last PV group with the finalize so
                            # the next qblock's scores enter the PE stream
                            # first (keeps ScalarE fed across the boundary)
                            last_pv = (ps_ctx, ex, c0, c1, j)
                        # interleave next qblock's Q projection mid-stream
                        if gi == 2 and j == 0 and qb + 1 < QC:
                            emit_qproj(qb + 1, qtiles.pop(qb + 1))
                    pend = (j, qb, ps_ctx, last_pv)
            finalize(*pend)   # last qblock + second AllToAll

            # ---- phase D: output projection of the core's 512 rows ----
            ctx_p = [persist.tile([128, 4, 512], BF16, name=f"cxp{j}")
                     for j in range(HPC)]
            for j in range(HPC):
                ev = a2a_out[j].rearrange("(a two) p q -> a two p q", two=2)
                nc.sync.dma_start(ctx_p[j][0:64, :, :],
                                  ev[:, 0].rearrange("a p q -> p a q"))
                nc.sync.dma_start(ctx_p[j][64:128, :, :],
                                  ev[:, 1].rearrange("a p q -> p a q"))
                for qc in range(SQ // 128):
                    for eh in range(2):
                        es = slice(eh * 512, (eh + 1) * 512)
                        ps_o = ps_s.tile([128, EXP_GROUP, 512], F32,
                                         tag="s", name=f"pso{j}_{qc}_{eh}")
                        ps_o = ps_o[:, 0, :]
                        for a in range(4):
                            nc.tensor.matmul(
                                ps_o[:],
                                ctx_p[j][:, a, 128 * qc:128 * (qc + 1)],
                                wo_sb[:, 4 * j + a, es],
                                start=(a == 0),
                                stop=(j == 1 and a == 3))
                        if j == 0:
                            nc.tensor.matmul(ps_o[:], ones_bf[:, 0:128],
                                             bo_sb[:, es], start=False,
                                             stop=True)
                            nc.vector.tensor_copy(o_acc[:, qc, es], ps_o[:])
                        else:
                            o_sb = c_misc.tile([128, 512], F32, tag="osb")
                            eng = nc.vector if (qc + eh) % 2 == 0 else \
                                nc.gpsimd
                            eng.tensor_add(o_sb[:], o_acc[:, qc, es],
                                           ps_o[:])
                            nc.sync.dma_start(
                                out[128 * qc:128 * (qc + 1), es], o_sb[:])
                if j == 0:
                    # keep the PE clock ramped through the second AllToAll:
                    # idle gaps reset the p-state and would double the cost
                    # of the j=1 output projection on the tail
                    ka = ps_aux.tile([128, 512], F32, tag="t", name="ka")
                    for i in range(KEEPALIVE):
                        nc.tensor.matmul(ka[:], kT_all[0:64, 0:128],
                                         kT_all[0:64, 0:512],
                                         start=True, stop=True)

    nc.compile()
    return nc


def prepare(query, key, value, mask, Wq, bq, Wk, bk, Wv, bv, Wo, bo):
    """Host-side sharding/preprocessing + program build."""
    query = np.asarray(query)
    key = np.asarray(key)
    value = np.asarray(value)
    mask = np.asarray(mask)
    Wq, bq = np.asarray(Wq), np.asarray(bq)
    Wk, bk = np.asarray(Wk), np.asarray(bk)
    Wv, bv = np.asarray(Wv), np.asarray(bv)
    Wo, bo = np.asarray(Wo), np.asarray(bo)

    idx = np.nonzero(mask.reshape(-1) != 0)[0]
    n = int(idx.size)

    def pmajor(xT):
        # [1024, cols] feature-major -> [128, 8, cols] partition-major
        return np.ascontiguousarray(
            xT.reshape(8, 128, xT.shape[1]).transpose(1, 0, 2))

    q_p = pmajor(_bf16(query[0].T))
    kc_p = pmajor(_bf16(key[0, idx, :].T))
    vc_p = pmajor(_bf16(value[0, idx, :].T))

    wqT = _bf16(Wq.T)   # [1024 in, 1024 out]
    wkT = _bf16(Wk.T)
    wvT = _bf16(Wv.T)
    woT_r = Wo.T
    slots = []
    for j in range(HPC):
        for a in range(4):
            hA, hB = 4 * a + j, 4 * a + 2 + j
            slots.append(woT_r[64 * hA:64 * hA + 64, :])
            slots.append(woT_r[64 * hB:64 * hB + 64, :])
    wo_p = pmajor(_bf16(np.concatenate(slots, axis=0)))
    bo_r = _bf16(bo.reshape(1, D))
    ident = np.eye(128, dtype=ml_dtypes.bfloat16)

    nc = build_program(n)

    in_maps = []
    for m in range(N_CORES):
        sl = slice(m * 128, (m + 1) * 128)
        in_maps.append({
            "q_p": q_p,
            "kc_p": kc_p,
            "vc_p": vc_p,
            "wq_p": pmajor(np.ascontiguousarray(wqT[:, sl])),
            "wk_p": pmajor(np.ascontiguousarray(wkT[:, sl])),
            "wv_p": pmajor(np.ascontiguousarray(wvT[:, sl])),
            "wo_p": wo_p,
            "bq_m": np.ascontiguousarray(
                bq[sl].reshape(128, 1).astype(np.float32)),
            "bk_m": np.ascontiguousarray(
                bk[sl].reshape(128, 1).astype(np.float32)),
            "bv_r": _bf16(bv[sl].reshape(1, 128)),
            "bo_r": bo_r,
            "ident": ident,
        })

    return {"nc": nc, "in_maps": in_maps, "n": n}


def kernel(query, key, value, mask, Wq, bq, Wk, bk, Wv, bv, Wo, bo,
           _trace=False, _result_box=None):
    prep = prepare(query, key, value, mask, Wq, bq, Wk, bk, Wv, bv, Wo, bo)
    res = run_bass_kernel_spmd(prep["nc"], prep["in_maps"],
                               list(range(N_CORES)), trace=_trace)
    if _result_box is not None:
        _result_box.append(res)

    out = np.concatenate([res.results[m]["out"] for m in range(N_CORES)],
                         axis=0)
    return out.reshape(1, S, D).astype(np.float32)
